# revision 14
# baseline (speedup 1.0000x reference)
import numpy as np

import concourse.bass as bass
import concourse.tile as tile
from concourse import bacc, mybir
from concourse.bass_utils import run_bass_kernel_spmd
from contextlib import ExitStack

N_CORES = 8
B_FULL = 131072
F = 256
H = 64
B_CORE = B_FULL // N_CORES

FD = 1024
CHUNK = 2 * FD
TIMESTEPS = 10
PI = float(np.pi)

f32 = mybir.dt.float32
f32r = mybir.dt.float32r
f16 = mybir.dt.float16
bf16 = mybir.dt.bfloat16
Alu = mybir.AluOpType
Act = mybir.ActivationFunctionType

WP16_COLS = 512 * 3 + 128 * 3
WPR_COLS = 256

(V_INVTHR, V_B1T, V_BETA, V_C2BB, V_BOVTHR, V_BO3, V_BV1, V_BV2,
 V_NEG1, V_ZERO) = range(10)

_BUILD_CACHE: dict = {}

def _register_lif_ops():
    import concourse.dve_ops as dv
    from concourse.dve_spec import Spec, Src0, Src1, C0, One, lower
    from concourse.dve_uop import DveOpSpec

    if "LIF_STEP2_ANT" in dv._SUB_OPCODE_FOR_NAME:
        by_name = {op.name: op for op in dv.OPS}
        return by_name["LIF_STEP1_ANT"], by_name["LIF_STEP2_ANT"]

    def ref1(in0, in1, s0, s1, imm2):
        M = in0.astype(np.float32); c = in1.astype(np.float32)
        b = np.asarray(s0, np.float32)
        return (b * M + c - (M > 1.0)).astype(np.float32)

    def ref2(in0, in1, s0, s1, imm2):
        m1 = ref1(in0, in1, s0, s1, imm2)
        return ref1(m1, in1, s0, s1, imm2)

    m1 = Src0 * C0 + Src1 - (Src0 > One)
    spec1 = Spec(body=m1, reference=ref1)
    spec2 = Spec(body=m1 * C0 + Src1 - (m1 > One), reference=ref2)

    ops = []
    for name, spec in (("LIF_STEP1_ANT", spec1), ("LIF_STEP2_ANT", spec2)):
        row = max(dv._SUB_OPCODE_FOR_NAME.values()) + 1
        shas = {
            ver: DveOpSpec(name=name, opcode=row, uops=lower(spec, ver=ver),
                           rd1_en=True).sha(ver)
            for ver in ("v3", "v4")
        }
        op = dv.DveOp(name, spec, subdim=False, uops_sha=shas)
        dv.OPS.append(op)
        dv.CUSTOM_DVE_SPECS[name] = spec
        dv._SUB_OPCODE_FOR_NAME[name] = row
        ops.append(op)
    return ops


LIF1, LIF2 = _register_lif_ops()




ENGINE_PLAN = None


def _super_engines(n_super: int) -> list:
    if ENGINE_PLAN is not None and len(ENGINE_PLAN) == n_super:
        return list(ENGINE_PLAN)
    if n_super == 1:
        return ["D"]
    if n_super == 2:
        return ["D", "P"]
    out = []
    for s in range(n_super):
        out.append("D" if s % 2 == 0 else "P")
    out[-1] = 672
    return out


def _build(bcore: int) -> bass.Bass:
    if bcore in _BUILD_CACHE:
        return _BUILD_CACHE[bcore]
    assert bcore % CHUNK == 0
    n_super = bcore // CHUNK
    engines = _super_engines(n_super)
    nj = FD // 512

    nc = bacc.Bacc(
        "TRN2", target_bir_lowering=False, debug=False, num_devices=N_CORES
    )

    xh = nc.dram_tensor("xh", [F, bcore], f16, kind="ExternalInput")
    xl = nc.dram_tensor("xl", [F, bcore], f16, kind="ExternalInput")
    wp16 = nc.dram_tensor("wp16", [128, WP16_COLS], f16, kind="ExternalInput")
    wpr = nc.dram_tensor("wpr", [128, WPR_COLS], f32r, kind="ExternalInput")
    vecs = nc.dram_tensor("vecs", [128, 16], f32, kind="ExternalInput")

    aT = nc.dram_tensor("aT", [128, bcore // 2], bf16, kind="ExternalOutput")
    cT = nc.dram_tensor("cT", [128, bcore // 2], bf16, kind="ExternalOutput")

    with tile.TileContext(nc) as tc, ExitStack() as ctx:
        wpool = ctx.enter_context(tc.tile_pool(name="weights", bufs=1))
        xpool = ctx.enter_context(tc.tile_pool(name="x", bufs=3))
        dpool = ctx.enter_context(tc.tile_pool(name="deep", bufs=5))
        mpool = ctx.enter_context(tc.tile_pool(name="work", bufs=2))
        opool = ctx.enter_context(tc.tile_pool(name="outs", bufs=2))
        ps_a = ctx.enter_context(
            tc.tile_pool(name="ps_a", bufs=2, space=bass.MemorySpace.PSUM)
        )
        ps_c2 = ctx.enter_context(
            tc.tile_pool(name="ps_c2", bufs=1, space=bass.MemorySpace.PSUM)
        )
        ps_c3 = ctx.enter_context(
            tc.tile_pool(name="ps_c3", bufs=1, space=bass.MemorySpace.PSUM)
        )

        w16 = wpool.tile([128, WP16_COLS], f16, tag="wp16")
        nc.sync.dma_start(w16[:], wp16[:])
        wr = wpool.tile([128, WPR_COLS], f32r, tag="wpr")
        nc.sync.dma_start(wr[:], wpr[:])
        vtile = wpool.tile([128, 16], f32, tag="vecs")
        nc.sync.dma_start(vtile[:], vecs[:])

        w1h = w16[:, 0:512]
        w1l = w16[:, 512:1024]
        wv1h = w16[:, 1024:1536]
        w2sh = w16[:, 1536:1664]
        w2sl = w16[:, 1664:1792]
        wos = w16[:, 1792:1920]
        ident = wr[:, 0:128]
        wv2 = wr[:, 128:256]

        def vec(i):
            return vtile[:, i : i + 1]

        invthr = vec(V_INVTHR)
        b1t = vec(V_B1T)
        beta = vec(V_BETA)
        c2bb = vec(V_C2BB)
        bovthr = vec(V_BOVTHR)
        bo3 = vec(V_BO3)
        bv1 = vec(V_BV1)
        bv2 = vec(V_BV2)
        neg1 = vec(V_NEG1)
        zero = vec(V_ZERO)

        sup = [dict() for _ in range(n_super)]

        def phase_a(s):
            st = sup[s]
            a0 = s * CHUNK
            xht, xlt = [], []
            for k in range(4):
                r0 = (k % 2) * 128
                c0 = a0 + (k // 2) * FD
                th = xpool.tile([128, FD], f16, tag=f"xh{k}")
                nc.sync.dma_start(th[:], xh[r0 : r0 + 128, c0 : c0 + FD])
                xht.append(th)
                tl = xpool.tile([128, FD], f16, tag=f"xl{k}")
                nc.sync.dma_start(tl[:], xl[r0 : r0 + 128, c0 : c0 + FD])
                xlt.append(tl)
            c1ps = ps_a.tile([128, FD], f32, tag="a")
            for j in range(nj):
                out = c1ps[:, j * 512 : (j + 1) * 512]
                first = True
                for k in range(4):
                    wk = slice(k * 128, (k + 1) * 128)
                    rh = xht[k][:, j * 512 : (j + 1) * 512]
                    rl = xlt[k][:, j * 512 : (j + 1) * 512]
                    nc.tensor.matmul(out, w1h[:, wk], rh, start=first, stop=False)
                    first = False
                    nc.tensor.matmul(out, w1l[:, wk], rh, start=False, stop=False)
                    nc.tensor.matmul(out, w1h[:, wk], rl, start=False, stop=(k == 3))
            c1p = dpool.tile([128, FD], f32, tag="c1p")
            nc.scalar.activation(c1p[:], c1ps[:], Act.Identity, bias=b1t, scale=invthr)
            st["c1p"] = c1p
            v1ps = ps_a.tile([128, FD], f32, tag="a")
            for j in range(nj):
                out = v1ps[:, j * 512 : (j + 1) * 512]
                for k in range(4):
                    wk = slice(k * 128, (k + 1) * 128)
                    nc.tensor.matmul(
                        out,
                        wv1h[:, wk],
                        xht[k][:, j * 512 : (j + 1) * 512],
                        start=(k == 0),
                        stop=(k == 3),
                    )
            v1 = mpool.tile([128, FD], f32r, tag="v1s")
            nc.scalar.activation(v1[:], v1ps[:], Act.Relu, bias=bv1, scale=1.0)
            v2ps = ps_a.tile([128, FD], f32, tag="a")
            for j in range(nj):
                nc.tensor.matmul(
                    v2ps[:, j * 512 : (j + 1) * 512],
                    wv2,
                    v1[:, j * 512 : (j + 1) * 512],
                    start=True,
                    stop=True,
                )
            critic = opool.tile([128, FD], bf16, tag="critic")
            nc.scalar.activation(critic[:], v2ps[:], Act.Relu, bias=bv2, scale=1.0)
            nc.sync.dma_start(cT[:, s * FD : (s + 1) * FD], critic[:])

        def phase_b(s):
            st = sup[s]
            c1p = st["c1p"]
            M = dpool.tile([128, FD], f32, tag="M")
            w = dpool.tile([128, FD], f32, tag="w")
            nc.vector._custom_dve(LIF2, out=M[:], in0=c1p[:], in1=c1p[:], s0=beta)
            nc.vector._custom_dve(LIF2, out=w[:], in0=M[:], in1=c1p[:], s0=beta)
            nc.vector._custom_dve(LIF2, out=M[:], in0=w[:], in1=c1p[:], s0=beta)
            nc.vector._custom_dve(LIF2, out=w[:], in0=M[:], in1=c1p[:], s0=beta)
            nc.vector._custom_dve(LIF1, out=M[:], in0=w[:], in1=c1p[:], s0=beta)
            st["M"] = M

        def phase_c(s):
            st = sup[s]
            M = st["M"]
            spk10 = mpool.tile([128, FD], f16, tag="spk10")
            nc.vector.tensor_scalar(spk10[:], M[:], 1.0, None, Alu.is_gt)
            v = mpool.tile([128, FD], f32r, tag="v")
            nc.vector.tensor_scalar(v[:], M[:], beta, c2bb, Alu.mult, Alu.add)
            c2ps = ps_c2.tile([128, FD], f32, tag="c2")
            for j in range(nj):
                out = c2ps[:, j * 512 : (j + 1) * 512]
                sg = spk10[:, j * 512 : (j + 1) * 512]
                vv = v[:, j * 512 : (j + 1) * 512]
                nc.tensor.matmul(out, w2sh, sg, start=True, stop=False)
                nc.tensor.matmul(out, w2sl, sg, start=False, stop=False)
                nc.tensor.matmul(out, ident, vv, start=False, stop=True)
            sgn2 = mpool.tile([128, FD], f16, tag="sgn2")
            nc.scalar.activation(sgn2[:], c2ps[:], Act.Sign, bias=neg1, scale=1.0)
            m2s = mpool.tile([128, FD], f32r, tag="m2s")
            nc.scalar.activation(m2s[:], c2ps[:], Act.Identity, bias=zero, scale=bovthr)
            c3ps = ps_c3.tile([128, FD], f32, tag="c3")
            for j in range(nj):
                out = c3ps[:, j * 512 : (j + 1) * 512]
                nc.tensor.matmul(
                    out, wos, sgn2[:, j * 512 : (j + 1) * 512], start=True, stop=False
                )
                nc.tensor.matmul(
                    out, ident, m2s[:, j * 512 : (j + 1) * 512], start=False, stop=True
                )
            act0 = mpool.tile([128, FD], f32, tag="act0")
            nc.scalar.activation(act0[:], c3ps[:], Act.Tanh, bias=bo3, scale=1.0)
            actb = opool.tile([128, FD], bf16, tag="actb")
            nc.scalar.activation(actb[:], act0[:], Act.Copy, bias=0.0, scale=PI)
            nc.sync.dma_start(aT[:, s * FD : (s + 1) * FD], actb[:])

        LOOKAHEAD = 3
        for s in range(min(LOOKAHEAD, n_super)):
            phase_a(s)
        for s in range(n_super):
            phase_b(s)
            if s + LOOKAHEAD < n_super:
                phase_a(s + LOOKAHEAD)
            phase_c(s)

    nc.finalize()
    _BUILD_CACHE[bcore] = nc
    return nc


def _blockdiag2(w: np.ndarray) -> np.ndarray:
    out = np.zeros((128, 128), np.float32)
    out[0:64, 0:64] = w
    out[64:128, 64:128] = w
    return out


def _chunks(W: np.ndarray) -> np.ndarray:
    c = np.zeros((4, 128, 128), np.float32)
    c[0, :, 0:64] = W[:, 0:128].T
    c[1, :, 0:64] = W[:, 128:256].T
    c[2, :, 64:128] = W[:, 0:128].T
    c[3, :, 64:128] = W[:, 128:256].T
    return np.ascontiguousarray(np.concatenate(list(c), axis=1))


def _make_consts(W1, b1, W2, b2, Wo, bo, beta_in, thr_in, beta_out, Wv1, bv1, Wv2, bv2):
    W1 = np.asarray(W1, np.float64)
    Wv1 = np.asarray(Wv1, np.float64)
    beta_c = np.clip(np.asarray(beta_in, np.float64), 0.0, 1.0)
    thr = np.asarray(thr_in, np.float64)
    invthr = 1.0 / thr
    bov = float(np.clip(np.asarray(beta_out, np.float64), 0.0, 1.0)[0])

    W1h = W1.astype(np.float16).astype(np.float64)
    W1l = (W1 - W1h).astype(np.float16).astype(np.float64)
    Wv1h = Wv1.astype(np.float16).astype(np.float64)

    W2pp = invthr[:, None] * (np.asarray(W2, np.float64) - np.diag(thr))
    W2sh = W2pp.astype(np.float16).astype(np.float64)
    W2sl = (W2pp - W2sh).astype(np.float16).astype(np.float64)
    Wos = 0.5 * np.asarray(Wo, np.float64)

    c2bb = np.asarray(b2, np.float64) * invthr
    bo3 = np.asarray(bo, np.float64) + Wos.sum(axis=1)
    bovthr = bov * thr

    wp16 = np.concatenate(
        [
            _chunks(W1h.astype(np.float32)),
            _chunks(W1l.astype(np.float32)),
            _chunks(Wv1h.astype(np.float32)),
            _blockdiag2(W2sh.T.astype(np.float32)),
            _blockdiag2(W2sl.T.astype(np.float32)),
            _blockdiag2(Wos.T.astype(np.float32)),
        ],
        axis=1,
    ).astype(np.float16)
    assert wp16.shape == (128, WP16_COLS)

    wpr = np.concatenate(
        [np.eye(128, dtype=np.float32), _blockdiag2(np.asarray(Wv2, np.float32).T)],
        axis=1,
    )
    assert wpr.shape == (128, WPR_COLS)

    def st(vv):
        return np.tile(np.asarray(vv, np.float64), 2).astype(np.float32)

    vecs16 = np.zeros((128, 16), np.float32)
    for i, vv in enumerate(
        [invthr, b1 * invthr, beta_c, c2bb, bovthr, bo3, bv1, bv2]
    ):
        vecs16[:, i] = st(vv)
    vecs16[:, V_NEG1] = -1.0
    vecs16[:, V_ZERO] = 0.0

    return dict(
        wp16=np.ascontiguousarray(wp16),
        wpr=np.ascontiguousarray(wpr),
        vecs=np.ascontiguousarray(vecs16),
    )


LAST_RES = None


def _run(x, consts, bcore):
    global LAST_RES
    nc = _build(bcore)
    n_cores = x.shape[0] // bcore
    xT = np.asarray(x, np.float32).T
    xh = xT.astype(np.float16)
    xl = (xT - xh.astype(np.float32)).astype(np.float16)
    xh = np.ascontiguousarray(xh)
    xl = np.ascontiguousarray(xl)
    in_maps = []
    for c in range(n_cores):
        m = dict(consts)
        m["xh"] = np.ascontiguousarray(xh[:, c * bcore : (c + 1) * bcore])
        m["xl"] = np.ascontiguousarray(xl[:, c * bcore : (c + 1) * bcore])
        in_maps.append(m)
    res = run_bass_kernel_spmd(nc, in_maps, list(range(n_cores)))
    LAST_RES = res
    n_super = bcore // CHUNK
    actors, critics = [], []
    for r in res.results:
        for name, acc in (("aT", actors), ("cT", critics)):
            t = np.asarray(r[name], np.float32).reshape(2, 64, n_super, FD)
            acc.append(t.transpose(2, 0, 3, 1).reshape(bcore, 64))
    actor = np.concatenate(actors, axis=0)
    critic = np.concatenate(critics, axis=0)
    return actor, critic


def kernel(x, W1, b1, W2, b2, Wo, bo, beta_in, thr_in, beta_out, Wv1, bv1, Wv2, bv2):
    x = np.asarray(x, np.float32)
    consts = _make_consts(
        W1, b1, W2, b2, Wo, bo, beta_in, thr_in, beta_out, Wv1, bv1, Wv2, bv2
    )
    return _run(x, consts, B_CORE)


# revision 24
# speedup vs baseline: 1.0518x; 1.0518x over previous
import numpy as np

import concourse.bass as bass
import concourse.tile as tile
from concourse import bacc, mybir
from concourse.bass_utils import run_bass_kernel_spmd
from contextlib import ExitStack

N_CORES = 8
B_FULL = 131072
F = 256
H = 64
B_CORE = B_FULL // N_CORES

FD = 1024
CHUNK = 2 * FD
TIMESTEPS = 10
PI = float(np.pi)

f32 = mybir.dt.float32
f32r = mybir.dt.float32r
f16 = mybir.dt.float16
bf16 = mybir.dt.bfloat16
Alu = mybir.AluOpType
Act = mybir.ActivationFunctionType

WP16_COLS = 512 * 3 + 128 * 3
WPR_COLS = 256

(V_INVTHR, V_B1T, V_BETA, V_C2BB, V_BOVTHR, V_BO3, V_BV1, V_BV2,
 V_NEG1, V_ZERO, V_SPK10) = range(11)

_BUILD_CACHE: dict = {}

def _register_lif_ops():
    import concourse.dve_ops as dv
    from concourse.dve_spec import Spec, Src0, Src1, C0, C1, One, lower
    from concourse.dve_uop import DveOpSpec

    if "LIF_STEP2_ANT" in dv._SUB_OPCODE_FOR_NAME:
        by_name = {op.name: op for op in dv.OPS}
        return (by_name["LIF_STEP1_ANT"], by_name["LIF_STEP2_ANT"],
            by_name["LIF_STEP1V_ANT"])

    def ref1(in0, in1, s0, s1, imm2):
        M = in0.astype(np.float32); c = in1.astype(np.float32)
        b = np.asarray(s0, np.float32)
        return (b * M + c - (M > 1.0)).astype(np.float32)

    def ref2(in0, in1, s0, s1, imm2):
        m1 = ref1(in0, in1, s0, s1, imm2)
        return ref1(m1, in1, s0, s1, imm2)

    def ref1v(in0, in1, s0, s1, imm2):
        m1 = ref1(in0, in1, s0, s1, imm2)
        return (np.asarray(s0, np.float32) * m1 + np.asarray(s1, np.float32)
                ).astype(np.float32)

    m1 = Src0 * C0 + Src1 - (Src0 > One)
    spec1 = Spec(body=m1, reference=ref1)
    spec2 = Spec(body=m1 * C0 + Src1 - (m1 > One), reference=ref2)
    spec1v = Spec(body=m1 * C0 + C1, reference=ref1v)

    ops = []
    for name, spec in (("LIF_STEP1_ANT", spec1), ("LIF_STEP2_ANT", spec2),
                       ("LIF_STEP1V_ANT", spec1v)):
        row = max(dv._SUB_OPCODE_FOR_NAME.values()) + 1
        shas = {
            ver: DveOpSpec(name=name, opcode=row, uops=lower(spec, ver=ver),
                           rd1_en=True).sha(ver)
            for ver in ("v3", "v4")
        }
        op = dv.DveOp(name, spec, subdim=False, uops_sha=shas)
        dv.OPS.append(op)
        dv.CUSTOM_DVE_SPECS[name] = spec
        dv._SUB_OPCODE_FOR_NAME[name] = row
        ops.append(op)
    return ops


LIF1, LIF2, LIF1V = _register_lif_ops()




ENGINE_PLAN = None


def _super_engines(n_super: int) -> list:
    if ENGINE_PLAN is not None and len(ENGINE_PLAN) == n_super:
        return list(ENGINE_PLAN)
    if n_super == 1:
        return ["D"]
    if n_super == 2:
        return ["D", "P"]
    out = []
    for s in range(n_super):
        out.append("D" if s % 2 == 0 else "P")
    out[-1] = 672
    return out


def _build(bcore: int) -> bass.Bass:
    if bcore in _BUILD_CACHE:
        return _BUILD_CACHE[bcore]
    assert bcore % CHUNK == 0
    n_super = bcore // CHUNK
    engines = _super_engines(n_super)
    nj = FD // 512

    nc = bacc.Bacc(
        "TRN2", target_bir_lowering=False, debug=False, num_devices=N_CORES
    )

    xh = nc.dram_tensor("xh", [128, 2, bcore], f16, kind="ExternalInput")
    xl = nc.dram_tensor("xl", [128, 2, bcore], f16, kind="ExternalInput")
    wp16 = nc.dram_tensor("wp16", [128, WP16_COLS], f16, kind="ExternalInput")
    wpr = nc.dram_tensor("wpr", [128, WPR_COLS], f32r, kind="ExternalInput")
    vecs = nc.dram_tensor("vecs", [128, 16], f32, kind="ExternalInput")

    aT = nc.dram_tensor("aT", [128, bcore // 2], f16, kind="ExternalOutput")
    cT = nc.dram_tensor("cT", [128, bcore // 2], f16, kind="ExternalOutput")

    with tile.TileContext(nc) as tc, ExitStack() as ctx:
        wpool = ctx.enter_context(tc.tile_pool(name="weights", bufs=1))
        xpool = ctx.enter_context(tc.tile_pool(name="x", bufs=3))
        dpool = ctx.enter_context(tc.tile_pool(name="deep", bufs=3))
        mpool = ctx.enter_context(tc.tile_pool(name="work", bufs=3))
        opool = ctx.enter_context(tc.tile_pool(name="outs", bufs=2))
        ps_a = ctx.enter_context(
            tc.tile_pool(name="ps_a", bufs=2, space=bass.MemorySpace.PSUM)
        )
        ps_c2 = ctx.enter_context(
            tc.tile_pool(name="ps_c2", bufs=1, space=bass.MemorySpace.PSUM)
        )
        ps_c3 = ctx.enter_context(
            tc.tile_pool(name="ps_c3", bufs=1, space=bass.MemorySpace.PSUM)
        )

        w16 = wpool.tile([128, WP16_COLS], f16, tag="wp16")
        nc.sync.dma_start(w16[:], wp16[:])
        wr = wpool.tile([128, WPR_COLS], f32r, tag="wpr")
        nc.sync.dma_start(wr[:], wpr[:])
        vtile = wpool.tile([128, 16], f32, tag="vecs")
        nc.sync.dma_start(vtile[:], vecs[:])

        w1h = w16[:, 0:512]
        w1l = w16[:, 512:1024]
        wv1h = w16[:, 1024:1536]
        w2sh = w16[:, 1536:1664]
        w2sl = w16[:, 1664:1792]
        wos = w16[:, 1792:1920]
        ident = wr[:, 0:128]
        wv2 = wr[:, 128:256]

        def vec(i):
            return vtile[:, i : i + 1]

        invthr = vec(V_INVTHR)
        b1t = vec(V_B1T)
        beta = vec(V_BETA)
        c2bb = vec(V_C2BB)
        bovthr = vec(V_BOVTHR)
        bo3 = vec(V_BO3)
        bv1 = vec(V_BV1)
        bv2 = vec(V_BV2)
        neg1 = vec(V_NEG1)
        zero = vec(V_ZERO)
        spk10thr = vec(V_SPK10)

        sup = [dict() for _ in range(n_super)]

        def phase_dma(s):
            st = sup[s]
            a0 = s * CHUNK
            xht = xpool.tile([128, 4 * FD], f16, tag="xh")
            xlt = xpool.tile([128, 4 * FD], f16, tag="xl")
            for bb in range(2):
                csl = slice(a0 + bb * FD, a0 + (bb + 1) * FD)
                dsl = slice(bb * 2 * FD, (bb + 1) * 2 * FD)
                nc.sync.dma_start(xht[:, dsl], xh[:, :, csl])
                nc.gpsimd.dma_start(xlt[:, dsl], xl[:, :, csl])
            st["xht"] = xht
            st["xlt"] = xlt

        def phase_a(s):
            st = sup[s]
            xht = st["xht"]
            xlt = st["xlt"]

            def xs(plane, m, j):
                return plane[:, m * FD + j * 512 : m * FD + (j + 1) * 512]

            c1ps = ps_a.tile([128, FD], f32, tag="a")
            cnt = [0, 0]

            def mm(out_ps, wsl, rhs, j, total):
                nc.tensor.matmul(
                    out_ps[:, j * 512 : (j + 1) * 512], wsl, rhs,
                    start=(cnt[j] == 0), stop=(cnt[j] == total - 1),
                )
                cnt[j] += 1

            for m in range(4):
                wk = slice(m * 128, (m + 1) * 128)
                for j in range(nj):
                    mm(c1ps, w1h[:, wk], xs(xht, m, j), j, 12)
                for j in range(nj):
                    mm(c1ps, w1h[:, wk], xs(xlt, m, j), j, 12)
                for j in range(nj):
                    mm(c1ps, w1l[:, wk], xs(xht, m, j), j, 12)
            c1p = dpool.tile([128, FD], f32, tag="c1p")
            nc.scalar.activation(c1p[:], c1ps[:], Act.Identity, bias=b1t, scale=invthr)
            st["c1p"] = c1p
            v1ps = ps_a.tile([128, FD], f32, tag="a")
            cnt = [0, 0]
            for m in range(4):
                wk = slice(m * 128, (m + 1) * 128)
                for j in range(nj):
                    mm(v1ps, wv1h[:, wk], xs(xht, m, j), j, 4)
            v1 = mpool.tile([128, FD], f32r, tag="v1s")
            nc.scalar.activation(v1[:], v1ps[:], Act.Relu, bias=bv1, scale=1.0)
            v2ps = ps_a.tile([128, FD], f32, tag="a")
            for j in range(nj):
                nc.tensor.matmul(
                    v2ps[:, j * 512 : (j + 1) * 512],
                    wv2,
                    v1[:, j * 512 : (j + 1) * 512],
                    start=True,
                    stop=True,
                )
            critic = opool.tile([128, FD], f16, tag="critic")
            nc.scalar.activation(critic[:], v2ps[:], Act.Relu, bias=bv2, scale=1.0)
            nc.sync.dma_start(cT[:, s * FD : (s + 1) * FD], critic[:])

        def phase_b(s):
            st = sup[s]
            c1p = st["c1p"]
            M = dpool.tile([128, FD], f32, tag="M")
            w = dpool.tile([128, FD], f32, tag="w")
            nc.vector._custom_dve(LIF2, out=M[:], in0=c1p[:], in1=c1p[:], s0=beta)
            nc.vector._custom_dve(LIF2, out=w[:], in0=M[:], in1=c1p[:], s0=beta)
            nc.vector._custom_dve(LIF2, out=M[:], in0=w[:], in1=c1p[:], s0=beta)
            nc.vector._custom_dve(LIF2, out=w[:], in0=M[:], in1=c1p[:], s0=beta)
            v = dpool.tile([128, FD], f32, tag="v")
            nc.vector._custom_dve(LIF1V, out=v[:], in0=w[:], in1=c1p[:], s0=beta,
                                  s1=c2bb)
            st["v"] = v

        def phase_c(s):
            st = sup[s]
            v = st["v"]
            spk10 = mpool.tile([128, FD], f16, tag="spk10")
            nc.vector.tensor_scalar(spk10[:], v[:], spk10thr, None, Alu.is_gt)
            c2ps = ps_c2.tile([128, FD], f32, tag="c2")
            for wsl, first, last in ((w2sh, True, False), (w2sl, False, True)):
                for j in range(nj):
                    nc.tensor.matmul(
                        c2ps[:, j * 512 : (j + 1) * 512],
                        wsl,
                        spk10[:, j * 512 : (j + 1) * 512],
                        start=first,
                        stop=last,
                    )
            M2 = mpool.tile([128, FD], f32, tag="M2")
            nc.vector.scalar_tensor_tensor(
                M2[:], c2ps[:], 1.0, v[:], Alu.mult, Alu.add
            )
            sgn2 = mpool.tile([128, FD], f16, tag="sgn2")
            nc.scalar.activation(sgn2[:], M2[:], Act.Sign, bias=neg1, scale=1.0)
            m2s = mpool.tile([128, FD], f32r, tag="m2s")
            nc.scalar.activation(m2s[:], M2[:], Act.Identity, bias=zero, scale=bovthr)
            c3ps = ps_c3.tile([128, FD], f32, tag="c3")
            for wsl, rhs, first, last in (
                (wos, sgn2, True, False),
                (ident, m2s, False, True),
            ):
                for j in range(nj):
                    nc.tensor.matmul(
                        c3ps[:, j * 512 : (j + 1) * 512],
                        wsl,
                        rhs[:, j * 512 : (j + 1) * 512],
                        start=first,
                        stop=last,
                    )
            act0 = mpool.tile([128, FD], f32, tag="act0")
            nc.scalar.activation(act0[:], c3ps[:], Act.Tanh, bias=bo3, scale=1.0)
            actb = opool.tile([128, FD], f16, tag="actb")
            nc.gpsimd.tensor_scalar(actb[:], act0[:], PI, None, Alu.mult)
            nc.sync.dma_start(aT[:, s * FD : (s + 1) * FD], actb[:])

        LOOKAHEAD = 3
        for s in range(min(LOOKAHEAD, n_super)):
            phase_a(s)
        for s in range(n_super):
            phase_b(s)
            if s + LOOKAHEAD < n_super:
                phase_a(s + LOOKAHEAD)
            if s >= 1:
                phase_c(s - 1)
        phase_c(n_super - 1)

    nc.finalize()
    _BUILD_CACHE[bcore] = nc
    return nc


def _blockdiag2(w: np.ndarray) -> np.ndarray:
    out = np.zeros((128, 128), np.float32)
    out[0:64, 0:64] = w
    out[64:128, 64:128] = w
    return out


def _chunks(W: np.ndarray) -> np.ndarray:
    c = np.zeros((4, 128, 128), np.float32)
    c[0, :, 0:64] = W[:, 0:128].T
    c[1, :, 0:64] = W[:, 128:256].T
    c[2, :, 64:128] = W[:, 0:128].T
    c[3, :, 64:128] = W[:, 128:256].T
    return np.ascontiguousarray(np.concatenate(list(c), axis=1))


def _make_consts(W1, b1, W2, b2, Wo, bo, beta_in, thr_in, beta_out, Wv1, bv1, Wv2, bv2):
    W1 = np.asarray(W1, np.float64)
    Wv1 = np.asarray(Wv1, np.float64)
    beta_c = np.clip(np.asarray(beta_in, np.float64), 0.0, 1.0)
    thr = np.asarray(thr_in, np.float64)
    invthr = 1.0 / thr
    bov = float(np.clip(np.asarray(beta_out, np.float64), 0.0, 1.0)[0])

    W1h = W1.astype(np.float16).astype(np.float64)
    W1l = (W1 - W1h).astype(np.float16).astype(np.float64)
    Wv1h = Wv1.astype(np.float16).astype(np.float64)

    W2pp = invthr[:, None] * (np.asarray(W2, np.float64) - np.diag(thr))
    W2sh = W2pp.astype(np.float16).astype(np.float64)
    W2sl = (W2pp - W2sh).astype(np.float16).astype(np.float64)
    Wos = 0.5 * np.asarray(Wo, np.float64)

    c2bb = np.asarray(b2, np.float64) * invthr
    bo3 = np.asarray(bo, np.float64) + Wos.sum(axis=1)
    bovthr = bov * thr

    wp16 = np.concatenate(
        [
            _chunks(W1h.astype(np.float32)),
            _chunks(W1l.astype(np.float32)),
            _chunks(Wv1h.astype(np.float32)),
            _blockdiag2(W2sh.T.astype(np.float32)),
            _blockdiag2(W2sl.T.astype(np.float32)),
            _blockdiag2(Wos.T.astype(np.float32)),
        ],
        axis=1,
    ).astype(np.float16)
    assert wp16.shape == (128, WP16_COLS)

    wpr = np.concatenate(
        [np.eye(128, dtype=np.float32), _blockdiag2(np.asarray(Wv2, np.float32).T)],
        axis=1,
    )
    assert wpr.shape == (128, WPR_COLS)

    def st(vv):
        return np.tile(np.asarray(vv, np.float64), 2).astype(np.float32)

    vecs16 = np.zeros((128, 16), np.float32)
    for i, vv in enumerate(
        [invthr, b1 * invthr, beta_c, c2bb, bovthr, bo3, bv1, bv2]
    ):
        vecs16[:, i] = st(vv)
    vecs16[:, V_NEG1] = -1.0
    vecs16[:, V_ZERO] = 0.0
    vecs16[:, V_SPK10] = st(beta_c + b2 * invthr)

    return dict(
        wp16=np.ascontiguousarray(wp16),
        wpr=np.ascontiguousarray(wpr),
        vecs=np.ascontiguousarray(vecs16),
    )


LAST_RES = None


def _prep_in_maps(x, consts, bcore):
    n_cores = x.shape[0] // bcore
    xT = np.asarray(x, np.float32).T
    xh = xT.astype(np.float16)
    xl = (xT - xh.astype(np.float32)).astype(np.float16)
    B = xT.shape[1]
    xh = np.ascontiguousarray(xh.reshape(2, 128, B).transpose(1, 0, 2))
    xl = np.ascontiguousarray(xl.reshape(2, 128, B).transpose(1, 0, 2))
    in_maps = []
    for c in range(n_cores):
        m = dict(consts)
        m["xh"] = np.ascontiguousarray(xh[:, :, c * bcore : (c + 1) * bcore])
        m["xl"] = np.ascontiguousarray(xl[:, :, c * bcore : (c + 1) * bcore])
        in_maps.append(m)
    return in_maps


def _run(x, consts, bcore):
    global LAST_RES
    nc = _build(bcore)
    n_cores = x.shape[0] // bcore
    in_maps = _prep_in_maps(x, consts, bcore)
    res = run_bass_kernel_spmd(nc, in_maps, list(range(n_cores)))
    LAST_RES = res
    n_super = bcore // CHUNK
    actors, critics = [], []
    for r in res.results:
        for name, acc in (("aT", actors), ("cT", critics)):
            t = np.asarray(r[name], np.float32).reshape(2, 64, n_super, FD)
            acc.append(t.transpose(2, 0, 3, 1).reshape(bcore, 64))
    actor = np.concatenate(actors, axis=0)
    critic = np.concatenate(critics, axis=0)
    return actor, critic


def kernel(x, W1, b1, W2, b2, Wo, bo, beta_in, thr_in, beta_out, Wv1, bv1, Wv2, bv2):
    x = np.asarray(x, np.float32)
    consts = _make_consts(
        W1, b1, W2, b2, Wo, bo, beta_in, thr_in, beta_out, Wv1, bv1, Wv2, bv2
    )
    return _run(x, consts, B_CORE)
